# revision 15
# baseline (speedup 1.0000x reference)
"""Trainium2 Bass kernel for nn_Attention_78280073937702.

Dense transformer attention block (prefill, B=1, S=2048, H=4096, 32 heads,
head_dim=128, fp32) sharded tensor-parallel over heads across 8 NeuronCores
(4 heads per core), with an AllToAll reshard so o_proj is sequence-sharded.

Host side pre-transposes and pre-casts everything to bf16 so the device does
zero layout work:
  hid_t  [H, S]          hidden^T            (same on all cores)
  wqk_t  [H, 8*128]      W_pack^T q/k cols, order q0,k0,q1,k1,... (local heads)
  wtv_t  [H, 512]        W_pack^T v cols (local heads)
  wo_t   [16, 1024, 1024] full W_o^T retiled [oc*4+hh][src*128+p][o-col]
  cos_t/sin_t [128, S]   RoPE tables (from position_ids)

Device per core:
  1. V proj (v natural [s,d], bf16) -> v_d DRAM
  2. per head: Q proj, K proj (PSUM fp32 -> bf16), RoPE (PE half-swap matmul
     + DVE mul/add, all-bf16 operands), kept in SBUF
  3. causal attention per head (S^T tiles -> exp bf16 -> mask -> esum bf16
     tree -> Z via ones-matmul colsum -> U^T = V E in PSUM -> attn^T bf16),
     pipelined with the next head's projection
  4. per head AllToAll resharding attn^T from head-sharded to seq-sharded
     (each core ends with all 4096 head-rows x its 256 seq cols)
  5. o_proj: out[s, o] natural layout, stationary = attn^T tiles, moving =
     streamed full W_o^T -> out_s [256, 4096] fp32

Host concatenates the 8 out_s slices along s. No transposes anywhere on host
output path.
"""

import os
import sys
from contextlib import ExitStack

import numpy as np
import ml_dtypes

for _p in ("/opt/trn_rl_repo", os.path.expanduser("~/.axon_site/_ro/trn_rl_repo")):
    if os.path.isdir(_p) and _p not in sys.path:
        sys.path.insert(0, _p)

import concourse.bacc as bacc  # noqa: E402
import concourse.bass as bass  # noqa: E402
import concourse.mybir as mybir  # noqa: E402
import concourse.tile as tile  # noqa: E402
from concourse.alu_op_type import AluOpType  # noqa: E402
from concourse.bass_utils import run_bass_kernel_spmd  # noqa: E402

F32 = mybir.dt.float32
BF16 = mybir.dt.bfloat16
EXPF = mybir.ActivationFunctionType.Exp

N_CORES = 8
S = 2048
H = 4096
D = 128
P = 128
N_HEADS = 32
NH_LOC = N_HEADS // N_CORES  # 4 heads per core
HT = H // P  # 32 h-tiles
ST = S // P  # 16 s-tiles
SL = 512  # s-slice width for matmul free dim
NSL = S // SL  # 4
VC = NH_LOC * D  # 512 local v columns
SC = S // N_CORES  # 256 seq cols per core after reshard
OCC = 1024  # o-column chunk for o_proj weight streaming
NOC = H // OCC  # 4
NORM = 1.0 / float(np.sqrt(D))


def build_nc():
    nc = bacc.Bacc("TRN2", target_bir_lowering=False, num_devices=N_CORES)

    hid_d = nc.dram_tensor("hid_t", [H, S], BF16, kind="ExternalInput")
    wqk_d = nc.dram_tensor("wqk_t", [H, 2 * NH_LOC * P], BF16, kind="ExternalInput")
    wtv_d = nc.dram_tensor("wtv_t", [H, VC], BF16, kind="ExternalInput")
    wo_d = nc.dram_tensor("wo_t", [NOC * 4, 8 * P, OCC], BF16, kind="ExternalInput")
    cos_d = nc.dram_tensor("cos_t", [D, S], BF16, kind="ExternalInput")
    sin_d = nc.dram_tensor("sin_t", [D, S], BF16, kind="ExternalInput")
    out_d = nc.dram_tensor("out_s", [SC, H], F32, kind="ExternalOutput")

    with tile.TileContext(nc) as tc, ExitStack() as ctx:
        dram = ctx.enter_context(tc.tile_pool(name="dram", bufs=1, space="DRAM"))
        v_d = dram.tile([ST, P, VC], BF16)
        attn_loc = [
            dram.tile([N_CORES, P, SC], BF16, name=f"aloc{h}")
            for h in range(NH_LOC)
        ]
        attn_recv = [
            dram.tile([N_CORES, P, SC], BF16, name=f"arecv{h}")
            for h in range(NH_LOC)
        ]

        # ---------------- constants ----------------
        consts = ctx.enter_context(tc.tile_pool(name="consts", bufs=1))
        ones_t = consts.tile([P, P], F32)
        nc.gpsimd.memset(ones_t, 1.0)
        # upper-triangular-with-diag keep-mask in [k, q] layout: keep q >= k
        tri01 = consts.tile([P, P], F32)
        nc.gpsimd.affine_select(
            out=tri01, in_=ones_t, compare_op=AluOpType.is_ge,
            fill=0.0, base=0, channel_multiplier=-1, pattern=[[1, P]],
        )
        tri01_b = consts.tile([P, P], BF16)
        nc.vector.tensor_copy(tri01_b, tri01)
        ones_b = consts.tile([P, P], BF16)
        nc.vector.tensor_copy(ones_b, ones_t)
        # signed half-swap as lhsT: lhsT[i, i+64] = +1 (i<64), [i, i-64] = -1
        neg_t = consts.tile([P, P], F32)
        nc.gpsimd.memset(neg_t, -1.0)
        sw_pos = consts.tile([P, P], F32)
        nc.gpsimd.affine_select(
            out=sw_pos, in_=ones_t, compare_op=AluOpType.is_equal,
            fill=0.0, base=-64, channel_multiplier=-1, pattern=[[1, P]],
        )
        sw_neg = consts.tile([P, P], F32)
        nc.gpsimd.affine_select(
            out=sw_neg, in_=neg_t, compare_op=AluOpType.is_equal,
            fill=0.0, base=64, channel_multiplier=-1, pattern=[[1, P]],
        )
        p_swap = consts.tile([P, P], F32)
        nc.vector.tensor_add(p_swap, sw_pos, sw_neg)
        p_swap_b = consts.tile([P, P], BF16)
        nc.vector.tensor_copy(p_swap_b, p_swap)

        with ExitStack() as ab:  # projection + attention phase
            hidT_pool = ab.enter_context(tc.tile_pool(name="hidT", bufs=1))
            hidT = hidT_pool.tile([P, HT, S], BF16)  # 128 KB/part

            trig = ab.enter_context(tc.tile_pool(name="trig", bufs=1))
            cosT = trig.tile([D, S], BF16)
            sinT = trig.tile([D, S], BF16)

            wqk_pool = ab.enter_context(tc.tile_pool(name="wqk", bufs=2))

            # ---------------- phase A: V projection ----------------
            with ExitStack() as vblk:
                wtv_pool = vblk.enter_context(tc.tile_pool(name="wtv", bufs=1))
                vps_pool = vblk.enter_context(
                    tc.tile_pool(name="vpsum", bufs=2, space="PSUM"))
                vstage = vblk.enter_context(tc.tile_pool(name="vstage", bufs=2))
                wtv = wtv_pool.tile([P, HT, VC], BF16)  # 32 KB/part
                # DMA emission order = first-consumption order; wide DMAs so
                # the framework fans them across many HW queues.  Stores are
                # issued from the ACT ring (nc.scalar) so they never queue
                # behind these loads.
                for g in range(4):
                    nc.sync.dma_start(
                        wtv[:, g * 8:(g + 1) * 8, :],
                        wtv_d[g * 8 * P:(g + 1) * 8 * P, :].rearrange(
                            "(t p) c -> p t c", p=P))
                for c in range(NSL):
                    if c == 0:  # split first chunk so V matmuls start early
                        for g in range(8):
                            nc.sync.dma_start(
                                hidT[:, g * 4:(g + 1) * 4, :SL],
                                hid_d[g * 4 * P:(g + 1) * 4 * P, :SL]
                                .rearrange("(t p) s -> p t s", p=P),
                            )
                    else:
                        nc.sync.dma_start(
                            hidT[:, :, c * SL:(c + 1) * SL],
                            hid_d[:, c * SL:(c + 1) * SL].rearrange(
                                "(t p) s -> p t s", p=P),
                        )
                nc.sync.dma_start(cosT, cos_d[:, :])
                nc.sync.dma_start(sinT, sin_d[:, :])
                for st in range(ST):
                    vps = vps_pool.tile([P, VC], F32, tag="v")
                    for ht in range(HT):
                        nc.tensor.matmul(
                            vps, hidT[:, ht, st * P:(st + 1) * P],
                            wtv[:, ht, :],
                            start=(ht == 0), stop=(ht == HT - 1),
                        )
                    vsb = vstage.tile([P, VC], BF16, tag="vs")
                    nc.scalar.copy(vsb, vps)
                    nc.scalar.dma_start(v_d[st], vsb)

            # ---------------- phase B+C: Q/K proj + RoPE + attention ------
            rstage = ab.enter_context(tc.tile_pool(name="rstage", bufs=2))
            qk_keep = ab.enter_context(tc.tile_pool(name="qkkeep", bufs=2))
            v_pool = ab.enter_context(tc.tile_pool(name="vio", bufs=2))
            e_pool = ab.enter_context(tc.tile_pool(name="epool", bufs=8))
            z_pool = ab.enter_context(tc.tile_pool(name="zpool", bufs=2))
            att_pool = ab.enter_context(tc.tile_pool(name="attst", bufs=2))
            qkps_pool = ab.enter_context(
                tc.tile_pool(name="qkpsum", bufs=1, space="PSUM"))
            rps_pool = ab.enter_context(
                tc.tile_pool(name="ropepsum", bufs=1, space="PSUM"))
            st_ps_pool = ab.enter_context(
                tc.tile_pool(name="stpsum", bufs=2, space="PSUM"))
            u_ps_pool = ab.enter_context(
                tc.tile_pool(name="upsum", bufs=2, space="PSUM"))
            z_ps_pool = ab.enter_context(
                tc.tile_pool(name="zpsum", bufs=1, space="PSUM"))

            def emit_attention(h, qk):
                vt = v_pool.tile([P, ST, P], BF16, tag="v")
                nc.sync.dma_start(
                    vt, v_d[:, :, h * P:(h + 1) * P].rearrange("t p d -> p t d"))
                qT = qk[:, 0, :]
                kT = qk[:, 1, :]
                for j in range(NSL):
                    nkt = 4 * j + 4
                    etiles = []
                    esum = z_pool.tile([P, SL], BF16, tag="es")
                    u_ps = u_ps_pool.tile([P, SL], F32, tag="u")
                    for i in range(nkt):
                        r = i - 4 * j
                        off = max(0, r) * P
                        et = e_pool.tile([P, SL], BF16, tag="e")
                        st_ps = st_ps_pool.tile([P, SL], F32, tag="st")
                        nc.tensor.matmul(
                            st_ps[:, off:],
                            kT[:, i * P:(i + 1) * P],
                            qT[:, j * SL + off:(j + 1) * SL],
                            start=True, stop=True,
                        )
                        nc.scalar.activation(
                            et[:, off:], st_ps[:, off:], EXPF, scale=NORM)
                        if r >= 0:
                            nc.vector.tensor_tensor(
                                et[:, off:off + P], et[:, off:off + P],
                                tri01_b, AluOpType.mult)
                        if i == 0:
                            nc.vector.tensor_copy(esum, et)
                        else:
                            nc.vector.tensor_tensor(
                                esum[:, off:], esum[:, off:], et[:, off:],
                                AluOpType.add)
                        etiles.append(et)
                        # interleave U matmuls 3 behind S matmuls
                        if i >= 3:
                            ui = i - 3
                            uoff = max(0, ui - 4 * j) * P
                            nc.tensor.matmul(
                                u_ps[:, uoff:], vt[:, ui, :],
                                etiles[ui][:, uoff:],
                                start=(ui == 0), stop=(ui == nkt - 1),
                            )
                    for ui in range(max(0, nkt - 3), nkt):
                        uoff = max(0, ui - 4 * j) * P
                        nc.tensor.matmul(
                            u_ps[:, uoff:], vt[:, ui, :], etiles[ui][:, uoff:],
                            start=(ui == 0), stop=(ui == nkt - 1),
                        )
                    zb_ps = z_ps_pool.tile([P, SL], F32, tag="zb")
                    nc.tensor.matmul(zb_ps, ones_b, esum, start=True, stop=True)
                    zr = z_pool.tile([P, SL], F32, tag="zr")
                    nc.vector.reciprocal(zr, zb_ps)
                    att = att_pool.tile([P, SL], BF16, tag="a")
                    nc.vector.tensor_tensor(att, u_ps, zr, AluOpType.mult)
                    nc.sync.dma_start(attn_loc[h][2 * j], att[:, :SC])
                    nc.sync.dma_start(attn_loc[h][2 * j + 1], att[:, SC:])
                # reshard this head now; overlaps remaining compute
                nc.gpsimd.collective_compute(
                    "AllToAll", AluOpType.bypass,
                    replica_groups=[list(range(N_CORES))],
                    ins=[attn_loc[h][:].opt()],
                    outs=[attn_recv[h][:].opt()],
                )

            qk = None
            for pt in range(2 * NH_LOC):  # q0,k0,q1,k1,...
                h, parity = pt // 2, pt % 2
                wqk = wqk_pool.tile([P, HT, P], BF16, tag="w")
                nc.sync.dma_start(
                    wqk,
                    wqk_d[:, pt * P:(pt + 1) * P].rearrange(
                        "(t p) c -> p t c", p=P),
                )
                if parity == 0:
                    qk = qk_keep.tile([P, 2, S], BF16, tag="qk")
                for slp in range(2):
                    qk_ps = [qkps_pool.tile([P, SL], F32, tag=f"qk{u}",
                                            name=f"qkps{u}")
                             for u in range(2)]
                    for ht in range(HT):
                        for u in range(2):
                            sl = slp * 2 + u
                            nc.tensor.matmul(
                                qk_ps[u], wqk[:, ht, :],
                                hidT[:, ht, sl * SL:(sl + 1) * SL],
                                start=(ht == 0), stop=(ht == HT - 1),
                            )
                    for u in range(2):
                        sl = slp * 2 + u
                        qt_b = rstage.tile([P, SL], BF16, tag="qt")
                        nc.scalar.copy(qt_b, qk_ps[u])
                        rps = rps_pool.tile([P, SL], F32, tag="r")
                        nc.tensor.matmul(rps, p_swap_b, qt_b,
                                         start=True, stop=True)
                        rps_b = rstage.tile([P, SL], BF16, tag="rb")
                        nc.scalar.copy(rps_b, rps)
                        t1 = rstage.tile([P, SL], BF16, tag="t1")
                        nc.vector.tensor_tensor(
                            t1, qt_b, cosT[:, sl * SL:(sl + 1) * SL],
                            AluOpType.mult)
                        t2 = rstage.tile([P, SL], BF16, tag="t2")
                        nc.vector.tensor_tensor(
                            t2, rps_b, sinT[:, sl * SL:(sl + 1) * SL],
                            AluOpType.mult)
                        nc.vector.tensor_tensor(
                            qk[:, parity, sl * SL:(sl + 1) * SL], t1, t2,
                            AluOpType.add)
                if parity == 1:
                    emit_attention(h, qk)

        # ---------------- phase E: o_proj (seq-sharded) ----------------
        with ExitStack() as e:
            at_pool = e.enter_context(tc.tile_pool(name="atT", bufs=1))
            wo_pool = e.enter_context(tc.tile_pool(name="wo", bufs=5))
            ops_pool = e.enter_context(
                tc.tile_pool(name="opsum", bufs=1, space="PSUM"))
            ostage = e.enter_context(tc.tile_pool(name="ostage", bufs=4))

            attnT = at_pool.tile([P, HT, SC], BF16)  # 16 KB/part
            for hh in range(NH_LOC):
                nc.sync.dma_start(
                    attnT[:, hh * 8:(hh + 1) * 8, :],
                    attn_recv[hh][:].rearrange("g p c -> p g c"),
                )

            def load_wo2(oc2):  # one 512-col o-chunk [P, HT, SL]
                oc, u = oc2 // 2, oc2 % 2
                t = wo_pool.tile([P, HT, SL], BF16, tag="wo")
                for hh in range(NH_LOC):
                    nc.sync.dma_start(
                        t[:, hh * 8:(hh + 1) * 8, :],
                        wo_d[oc * 4 + hh][:, u * SL:(u + 1) * SL].rearrange(
                            "(g p) c -> p g c", p=P),
                    )
                return t

            # 8 single 512-col chunks, 4 accumulating concurrently
            # (2 PSUM banks each).  The recv3-gated k-tiles (t >= 24) of the
            # first 4 chunks are deferred so most of o_proj can execute
            # before the last AllToAll lands.
            wt = [load_wo2(i) for i in range(4)]
            ops = {}

            def mk_ops(c):
                ops[c] = [ops_pool.tile([P, SL], F32, tag=f"o{c % 4}_{s}",
                                        name=f"ops{c % 4}_{s}")
                          for s in range(2)]

            def chunk_mms(oc2, t0, t1):
                for t in range(t0, t1):
                    for st_ in range(2):
                        nc.tensor.matmul(
                            ops[oc2][st_],
                            attnT[:, t, st_ * P:(st_ + 1) * P],
                            wt[oc2][:, t, :],
                            start=(t == 0), stop=(t == HT - 1),
                        )

            def drain(oc2):
                for st_ in range(2):
                    ob = ostage.tile([P, SL], F32, tag="ob")
                    nc.scalar.copy(ob, ops[oc2][st_])
                    nc.scalar.dma_start(
                        out_d[st_ * P:(st_ + 1) * P,
                              oc2 * SL:(oc2 + 1) * SL],
                        ob,
                    )

            for c in range(4):
                mk_ops(c)
                chunk_mms(c, 0, 24)
                if c == 0:
                    wt.append(load_wo2(4))  # 5th wo buffer
            for c in range(4):
                chunk_mms(c, 24, HT)
                drain(c)
                if c + 5 < 8:
                    wt.append(load_wo2(c + 5))
                c2 = c + 4
                mk_ops(c2)
                chunk_mms(c2, 0, HT)
                drain(c2)

    nc.compile()
    return nc


def make_in_maps(hidden_states, position_ids, W_pack, W_o):
    bf = ml_dtypes.bfloat16
    hidden = np.asarray(hidden_states, dtype=np.float32).reshape(S, H)
    W_pack = np.asarray(W_pack, dtype=np.float32)
    W_o = np.asarray(W_o, dtype=np.float32)
    pos = np.asarray(position_ids).reshape(S).astype(np.float64)

    hid_t = np.ascontiguousarray(hidden.T).astype(bf)  # [H, S]

    inv_freq = 1.0 / (10000.0 ** (np.arange(0, D, 2, dtype=np.float64) / D))
    freqs = np.outer(pos, inv_freq)  # [S, D/2]
    emb = np.concatenate([freqs, freqs], axis=1)  # [S, D]
    cos_t = np.ascontiguousarray(np.cos(emb).T).astype(bf)  # [D, S]
    sin_t = np.ascontiguousarray(np.sin(emb).T).astype(bf)

    # full W_o^T retiled: wo_t[oc*4+hh, src*128+p, c] =
    #   W_o[oc*1024+c, (src*4+hh)*128+p]
    woT = np.ascontiguousarray(W_o.T)  # [h', o]
    wo_t = np.empty((NOC * 4, 8 * P, OCC), dtype=bf)
    for oc in range(NOC):
        for hh in range(NH_LOC):
            for src in range(N_CORES):
                g = src * NH_LOC + hh
                wo_t[oc * 4 + hh, src * P:(src + 1) * P, :] = \
                    woT[g * P:(g + 1) * P, oc * OCC:(oc + 1) * OCC].astype(bf)

    in_maps = []
    for c in range(N_CORES):
        cols = []
        for hh in range(NH_LOC):
            q_rows = W_pack[c * VC + hh * P:c * VC + (hh + 1) * P]
            k_rows = W_pack[H + c * VC + hh * P:H + c * VC + (hh + 1) * P]
            cols.append(q_rows)
            cols.append(k_rows)
        wqk = np.concatenate(cols, axis=0)  # [1024, H] rows in pt order
        wqk_t = np.ascontiguousarray(wqk.T).astype(bf)  # [H, 1024]
        wtv_t = np.ascontiguousarray(
            W_pack[2 * H + c * VC:2 * H + (c + 1) * VC].T).astype(bf)
        in_maps.append({
            "hid_t": hid_t,
            "wqk_t": wqk_t,
            "wtv_t": wtv_t,
            "wo_t": wo_t,
            "cos_t": cos_t,
            "sin_t": sin_t,
        })
    return in_maps


_NC_CACHE = None


def get_nc():
    global _NC_CACHE
    if _NC_CACHE is None:
        _NC_CACHE = build_nc()
    return _NC_CACHE


def run(inputs, trace=False):
    """Run on hardware; returns (output [1,S,H] f32, BassKernelResults)."""
    in_maps = make_in_maps(
        inputs["hidden_states"], inputs["position_ids"],
        inputs["W_pack"], inputs["W_o"])
    nc = get_nc()
    res = run_bass_kernel_spmd(nc, in_maps, list(range(N_CORES)), trace=trace)
    parts = [np.asarray(res.results[c]["out_s"]) for c in range(N_CORES)]
    out = np.concatenate(parts, axis=0)[None]  # [1, S, H]
    return out.astype(np.float32), res


def kernel(**inputs):
    out, _ = run(inputs, trace=False)
    return out


# revision 17
# speedup vs baseline: 1.0569x; 1.0569x over previous
"""Trainium2 Bass kernel for nn_Attention_78280073937702.

Dense transformer attention block (prefill, B=1, S=2048, H=4096, 32 heads,
head_dim=128, fp32) sharded tensor-parallel over heads across 8 NeuronCores
(4 heads per core), with an AllToAll reshard so o_proj is sequence-sharded.

Host side pre-transposes and pre-casts everything to bf16 so the device does
zero layout work:
  hid_t  [H, S]          hidden^T            (same on all cores)
  wqk_t  [H, 8*128]      W_pack^T q/k cols, order q0,k0,q1,k1,... (local heads)
  wtv_t  [H, 512]        W_pack^T v cols (local heads)
  wo_t   [16, 1024, 1024] full W_o^T retiled [oc*4+hh][src*128+p][o-col]
  cos_t/sin_t [128, S]   RoPE tables (from position_ids)

Device per core:
  1. V proj (v natural [s,d], bf16) -> v_d DRAM
  2. per head: Q proj, K proj (PSUM fp32 -> bf16), RoPE (PE half-swap matmul
     + DVE mul/add, all-bf16 operands), kept in SBUF
  3. causal attention per head (S^T tiles -> exp bf16 -> mask -> esum bf16
     tree -> Z via ones-matmul colsum -> U^T = V E in PSUM -> attn^T bf16),
     pipelined with the next head's projection
  4. per head AllToAll resharding attn^T from head-sharded to seq-sharded
     (each core ends with all 4096 head-rows x its 256 seq cols)
  5. o_proj: out[s, o] natural layout, stationary = attn^T tiles, moving =
     streamed full W_o^T -> out_s [256, 4096] fp32

Host concatenates the 8 out_s slices along s. No transposes anywhere on host
output path.
"""

import os
import sys
from contextlib import ExitStack

import numpy as np
import ml_dtypes

for _p in ("/opt/trn_rl_repo", os.path.expanduser("~/.axon_site/_ro/trn_rl_repo")):
    if os.path.isdir(_p) and _p not in sys.path:
        sys.path.insert(0, _p)

import concourse.bacc as bacc  # noqa: E402
import concourse.bass as bass  # noqa: E402
import concourse.mybir as mybir  # noqa: E402
import concourse.tile as tile  # noqa: E402
from concourse.alu_op_type import AluOpType  # noqa: E402
from concourse.bass_utils import run_bass_kernel_spmd  # noqa: E402

F32 = mybir.dt.float32
BF16 = mybir.dt.bfloat16
EXPF = mybir.ActivationFunctionType.Exp

N_CORES = 8
S = 2048
H = 4096
D = 128
P = 128
N_HEADS = 32
NH_LOC = N_HEADS // N_CORES  # 4 heads per core
HT = H // P  # 32 h-tiles
ST = S // P  # 16 s-tiles
SL = 512  # s-slice width for matmul free dim
NSL = S // SL  # 4
VC = NH_LOC * D  # 512 local v columns
SC = S // N_CORES  # 256 seq cols per core after reshard
OCC = 1024  # o-column chunk for o_proj weight streaming
NOC = H // OCC  # 4
NORM = 1.0 / float(np.sqrt(D))


def build_nc():
    nc = bacc.Bacc("TRN2", target_bir_lowering=False, num_devices=N_CORES)

    hid_d = nc.dram_tensor("hid_t", [H, S], BF16, kind="ExternalInput")
    wqk_d = nc.dram_tensor("wqk_t", [H, 2 * NH_LOC * P], BF16, kind="ExternalInput")
    wtv_d = nc.dram_tensor("wtv_t", [H, VC], BF16, kind="ExternalInput")
    wo_d = nc.dram_tensor("wo_t", [NOC * 4, 8 * P, OCC], BF16, kind="ExternalInput")
    cos_d = nc.dram_tensor("cos_t", [D, S], BF16, kind="ExternalInput")
    sin_d = nc.dram_tensor("sin_t", [D, S], BF16, kind="ExternalInput")
    out_d = nc.dram_tensor("out_s", [SC, H], F32, kind="ExternalOutput")

    with tile.TileContext(nc) as tc, ExitStack() as ctx:
        dram = ctx.enter_context(tc.tile_pool(name="dram", bufs=1, space="DRAM"))
        v_d = dram.tile([ST, P, VC], BF16)
        attn_loc = [
            dram.tile([N_CORES, P, SC], BF16, name=f"aloc{h}")
            for h in range(NH_LOC)
        ]
        attn_recv = [
            dram.tile([N_CORES, P, SC], BF16, name=f"arecv{h}")
            for h in range(NH_LOC)
        ]

        # ---------------- constants ----------------
        consts = ctx.enter_context(tc.tile_pool(name="consts", bufs=1))
        ones_t = consts.tile([P, P], F32)
        nc.gpsimd.memset(ones_t, 1.0)
        # upper-triangular-with-diag keep-mask in [k, q] layout: keep q >= k
        tri01 = consts.tile([P, P], F32)
        nc.gpsimd.affine_select(
            out=tri01, in_=ones_t, compare_op=AluOpType.is_ge,
            fill=0.0, base=0, channel_multiplier=-1, pattern=[[1, P]],
        )
        tri01_b = consts.tile([P, P], BF16)
        nc.vector.tensor_copy(tri01_b, tri01)
        ones_b = consts.tile([P, P], BF16)
        nc.vector.tensor_copy(ones_b, ones_t)
        # signed half-swap as lhsT: lhsT[i, i+64] = +1 (i<64), [i, i-64] = -1
        neg_t = consts.tile([P, P], F32)
        nc.gpsimd.memset(neg_t, -1.0)
        sw_pos = consts.tile([P, P], F32)
        nc.gpsimd.affine_select(
            out=sw_pos, in_=ones_t, compare_op=AluOpType.is_equal,
            fill=0.0, base=-64, channel_multiplier=-1, pattern=[[1, P]],
        )
        sw_neg = consts.tile([P, P], F32)
        nc.gpsimd.affine_select(
            out=sw_neg, in_=neg_t, compare_op=AluOpType.is_equal,
            fill=0.0, base=64, channel_multiplier=-1, pattern=[[1, P]],
        )
        p_swap = consts.tile([P, P], F32)
        nc.vector.tensor_add(p_swap, sw_pos, sw_neg)
        p_swap_b = consts.tile([P, P], BF16)
        nc.vector.tensor_copy(p_swap_b, p_swap)

        with ExitStack() as ab:  # projection + attention phase
            hidT_pool = ab.enter_context(tc.tile_pool(name="hidT", bufs=1))
            hidT = hidT_pool.tile([P, HT, S], BF16)  # 128 KB/part

            trig = ab.enter_context(tc.tile_pool(name="trig", bufs=1))
            cosT = trig.tile([D, S], BF16)
            sinT = trig.tile([D, S], BF16)

            wqk_pool = ab.enter_context(tc.tile_pool(name="wqk", bufs=2))

            # ---------------- phase A: V projection ----------------
            with ExitStack() as vblk:
                wtv_pool = vblk.enter_context(tc.tile_pool(name="wtv", bufs=1))
                vps_pool = vblk.enter_context(
                    tc.tile_pool(name="vpsum", bufs=2, space="PSUM"))
                vstage = vblk.enter_context(tc.tile_pool(name="vstage", bufs=2))
                wtv = wtv_pool.tile([P, HT, VC], BF16)  # 32 KB/part
                # DMA emission order = first-consumption order; wide DMAs so
                # the framework fans them across many HW queues.  Stores are
                # issued from the ACT ring (nc.scalar) so they never queue
                # behind these loads.
                for g in range(4):
                    nc.sync.dma_start(
                        wtv[:, g * 8:(g + 1) * 8, :],
                        wtv_d[g * 8 * P:(g + 1) * 8 * P, :].rearrange(
                            "(t p) c -> p t c", p=P))
                for c in range(NSL):
                    if c == 0:  # split first chunk so V matmuls start early
                        for g in range(8):
                            nc.sync.dma_start(
                                hidT[:, g * 4:(g + 1) * 4, :SL],
                                hid_d[g * 4 * P:(g + 1) * 4 * P, :SL]
                                .rearrange("(t p) s -> p t s", p=P),
                            )
                    else:
                        nc.sync.dma_start(
                            hidT[:, :, c * SL:(c + 1) * SL],
                            hid_d[:, c * SL:(c + 1) * SL].rearrange(
                                "(t p) s -> p t s", p=P),
                        )
                nc.sync.dma_start(cosT, cos_d[:, :])
                nc.sync.dma_start(sinT, sin_d[:, :])
                for st in range(ST):
                    vps = vps_pool.tile([P, VC], F32, tag="v")
                    for ht in range(HT):
                        nc.tensor.matmul(
                            vps, hidT[:, ht, st * P:(st + 1) * P],
                            wtv[:, ht, :],
                            start=(ht == 0), stop=(ht == HT - 1),
                        )
                    vsb = vstage.tile([P, VC], BF16, tag="vs")
                    nc.scalar.copy(vsb, vps)
                    nc.scalar.dma_start(v_d[st], vsb)

            # ---------------- phase B+C: Q/K proj + RoPE + attention ------
            rstage = ab.enter_context(tc.tile_pool(name="rstage", bufs=2))
            qk_keep = ab.enter_context(tc.tile_pool(name="qkkeep", bufs=2))
            v_pool = ab.enter_context(tc.tile_pool(name="vio", bufs=2))
            e_pool = ab.enter_context(tc.tile_pool(name="epool", bufs=8))
            z_pool = ab.enter_context(tc.tile_pool(name="zpool", bufs=2))
            att_pool = ab.enter_context(tc.tile_pool(name="attst", bufs=2))
            qkps_pool = ab.enter_context(
                tc.tile_pool(name="qkpsum", bufs=1, space="PSUM"))
            rps_pool = ab.enter_context(
                tc.tile_pool(name="ropepsum", bufs=1, space="PSUM"))
            st_ps_pool = ab.enter_context(
                tc.tile_pool(name="stpsum", bufs=2, space="PSUM"))
            u_ps_pool = ab.enter_context(
                tc.tile_pool(name="upsum", bufs=2, space="PSUM"))
            z_ps_pool = ab.enter_context(
                tc.tile_pool(name="zpsum", bufs=1, space="PSUM"))

            # deferred z-chain: z/recip/att-mult/stores of block (h, j) are
            # emitted a few PE matmuls into the NEXT block so the PE never
            # waits on the DVE esum tree
            def flush(p):
                h, j, esum, u_ps = p
                zb_ps = z_ps_pool.tile([P, SL], F32, tag="zb")
                nc.tensor.matmul(zb_ps, ones_b, esum, start=True, stop=True)
                zr = z_pool.tile([P, SL], F32, tag="zr")
                nc.vector.reciprocal(zr, zb_ps)
                att = att_pool.tile([P, SL], BF16, tag="a")
                nc.vector.tensor_tensor(att, u_ps, zr, AluOpType.mult)
                nc.sync.dma_start(attn_loc[h][2 * j], att[:, :SC])
                nc.sync.dma_start(attn_loc[h][2 * j + 1], att[:, SC:])
                if j == NSL - 1:
                    # reshard this head now; overlaps remaining compute
                    nc.gpsimd.collective_compute(
                        "AllToAll", AluOpType.bypass,
                        replica_groups=[list(range(N_CORES))],
                        ins=[attn_loc[h][:].opt()],
                        outs=[attn_recv[h][:].opt()],
                    )

            def attn_j(h, qk, vt, j, pend):
                qT = qk[:, 0, :]
                kT = qk[:, 1, :]
                nkt = 4 * j + 4
                etiles = []
                esum = z_pool.tile([P, SL], BF16, tag="es")
                u_ps = u_ps_pool.tile([P, SL], F32, tag="u")
                for i in range(nkt):
                    r = i - 4 * j
                    off = max(0, r) * P
                    et = e_pool.tile([P, SL], BF16, tag="e")
                    st_ps = st_ps_pool.tile([P, SL], F32, tag="st")
                    nc.tensor.matmul(
                        st_ps[:, off:],
                        kT[:, i * P:(i + 1) * P],
                        qT[:, j * SL + off:(j + 1) * SL],
                        start=True, stop=True,
                    )
                    nc.scalar.activation(
                        et[:, off:], st_ps[:, off:], EXPF, scale=NORM)
                    if r >= 0:
                        nc.vector.tensor_tensor(
                            et[:, off:off + P], et[:, off:off + P],
                            tri01_b, AluOpType.mult)
                    if i == 0:
                        nc.vector.tensor_copy(esum, et)
                    else:
                        nc.vector.tensor_tensor(
                            esum[:, off:], esum[:, off:], et[:, off:],
                            AluOpType.add)
                    etiles.append(et)
                    if i == 2 and pend is not None:
                        flush(pend)
                        pend = None
                    # interleave U matmuls 3 behind S matmuls
                    if i >= 3:
                        ui = i - 3
                        uoff = max(0, ui - 4 * j) * P
                        nc.tensor.matmul(
                            u_ps[:, uoff:], vt[:, ui, :],
                            etiles[ui][:, uoff:],
                            start=(ui == 0), stop=(ui == nkt - 1),
                        )
                for ui in range(max(0, nkt - 3), nkt):
                    uoff = max(0, ui - 4 * j) * P
                    nc.tensor.matmul(
                        u_ps[:, uoff:], vt[:, ui, :], etiles[ui][:, uoff:],
                        start=(ui == 0), stop=(ui == nkt - 1),
                    )
                return (h, j, esum, u_ps)

            qk = None
            vt = None
            pend = None
            for pt in range(2 * NH_LOC):  # q0,k0,q1,k1,...
                h, parity = pt // 2, pt % 2
                wqk = wqk_pool.tile([P, HT, P], BF16, tag="w")
                nc.sync.dma_start(
                    wqk,
                    wqk_d[:, pt * P:(pt + 1) * P].rearrange(
                        "(t p) c -> p t c", p=P),
                )
                if parity == 0:
                    qk = qk_keep.tile([P, 2, S], BF16, tag="qk")
                    vt = v_pool.tile([P, ST, P], BF16, tag="v")
                    nc.sync.dma_start(
                        vt,
                        v_d[:, :, h * P:(h + 1) * P].rearrange("t p d -> p t d"))
                for slp in range(2):
                    qk_ps = [qkps_pool.tile([P, SL], F32, tag=f"qk{u}",
                                            name=f"qkps{u}")
                             for u in range(2)]
                    for ht in range(HT):
                        for u in range(2):
                            sl = slp * 2 + u
                            nc.tensor.matmul(
                                qk_ps[u], wqk[:, ht, :],
                                hidT[:, ht, sl * SL:(sl + 1) * SL],
                                start=(ht == 0), stop=(ht == HT - 1),
                            )
                        if ht == 2 and pend is not None:
                            flush(pend)
                            pend = None
                    for u in range(2):
                        sl = slp * 2 + u
                        qt_b = rstage.tile([P, SL], BF16, tag="qt")
                        nc.scalar.copy(qt_b, qk_ps[u])
                        rps = rps_pool.tile([P, SL], F32, tag="r")
                        nc.tensor.matmul(rps, p_swap_b, qt_b,
                                         start=True, stop=True)
                        rps_b = rstage.tile([P, SL], BF16, tag="rb")
                        nc.scalar.copy(rps_b, rps)
                        t1 = rstage.tile([P, SL], BF16, tag="t1")
                        nc.vector.tensor_tensor(
                            t1, qt_b, cosT[:, sl * SL:(sl + 1) * SL],
                            AluOpType.mult)
                        t2 = rstage.tile([P, SL], BF16, tag="t2")
                        nc.vector.tensor_tensor(
                            t2, rps_b, sinT[:, sl * SL:(sl + 1) * SL],
                            AluOpType.mult)
                        nc.vector.tensor_tensor(
                            qk[:, parity, sl * SL:(sl + 1) * SL], t1, t2,
                            AluOpType.add)
                    if parity == 1:
                        # attention blocks as soon as their k-slices are roped
                        j0 = 2 * slp
                        pend = attn_j(h, qk, vt, j0, pend)
                        pend = attn_j(h, qk, vt, j0 + 1, pend)
            flush(pend)  # last head's j=3

        # ---------------- phase E: o_proj (seq-sharded) ----------------
        with ExitStack() as e:
            at_pool = e.enter_context(tc.tile_pool(name="atT", bufs=1))
            wo_pool = e.enter_context(tc.tile_pool(name="wo", bufs=5))
            ops_pool = e.enter_context(
                tc.tile_pool(name="opsum", bufs=1, space="PSUM"))
            ostage = e.enter_context(tc.tile_pool(name="ostage", bufs=4))

            attnT = at_pool.tile([P, HT, SC], BF16)  # 16 KB/part
            for hh in range(NH_LOC):
                nc.sync.dma_start(
                    attnT[:, hh * 8:(hh + 1) * 8, :],
                    attn_recv[hh][:].rearrange("g p c -> p g c"),
                )

            def load_wo2(oc2):  # one 512-col o-chunk [P, HT, SL]
                oc, u = oc2 // 2, oc2 % 2
                t = wo_pool.tile([P, HT, SL], BF16, tag="wo")
                for hh in range(NH_LOC):
                    nc.sync.dma_start(
                        t[:, hh * 8:(hh + 1) * 8, :],
                        wo_d[oc * 4 + hh][:, u * SL:(u + 1) * SL].rearrange(
                            "(g p) c -> p g c", p=P),
                    )
                return t

            # 8 single 512-col chunks, 4 accumulating concurrently
            # (2 PSUM banks each).  The recv3-gated k-tiles (t >= 24) of the
            # first 4 chunks are deferred so most of o_proj can execute
            # before the last AllToAll lands.
            wt = [load_wo2(i) for i in range(4)]
            ops = {}

            def mk_ops(c):
                ops[c] = [ops_pool.tile([P, SL], F32, tag=f"o{c % 4}_{s}",
                                        name=f"ops{c % 4}_{s}")
                          for s in range(2)]

            def chunk_mms(oc2, t0, t1):
                for t in range(t0, t1):
                    for st_ in range(2):
                        nc.tensor.matmul(
                            ops[oc2][st_],
                            attnT[:, t, st_ * P:(st_ + 1) * P],
                            wt[oc2][:, t, :],
                            start=(t == 0), stop=(t == HT - 1),
                        )

            def drain(oc2):
                for st_ in range(2):
                    ob = ostage.tile([P, SL], F32, tag="ob")
                    nc.scalar.copy(ob, ops[oc2][st_])
                    nc.scalar.dma_start(
                        out_d[st_ * P:(st_ + 1) * P,
                              oc2 * SL:(oc2 + 1) * SL],
                        ob,
                    )

            for c in range(4):
                mk_ops(c)
                chunk_mms(c, 0, 24)
                if c == 0:
                    wt.append(load_wo2(4))  # 5th wo buffer
            for c in range(4):
                chunk_mms(c, 24, HT)
                drain(c)
                if c + 5 < 8:
                    wt.append(load_wo2(c + 5))
                c2 = c + 4
                mk_ops(c2)
                chunk_mms(c2, 0, HT)
                drain(c2)

    nc.compile()
    return nc


def make_in_maps(hidden_states, position_ids, W_pack, W_o):
    bf = ml_dtypes.bfloat16
    hidden = np.asarray(hidden_states, dtype=np.float32).reshape(S, H)
    W_pack = np.asarray(W_pack, dtype=np.float32)
    W_o = np.asarray(W_o, dtype=np.float32)
    pos = np.asarray(position_ids).reshape(S).astype(np.float64)

    hid_t = np.ascontiguousarray(hidden.T).astype(bf)  # [H, S]

    inv_freq = 1.0 / (10000.0 ** (np.arange(0, D, 2, dtype=np.float64) / D))
    freqs = np.outer(pos, inv_freq)  # [S, D/2]
    emb = np.concatenate([freqs, freqs], axis=1)  # [S, D]
    cos_t = np.ascontiguousarray(np.cos(emb).T).astype(bf)  # [D, S]
    sin_t = np.ascontiguousarray(np.sin(emb).T).astype(bf)

    # full W_o^T retiled: wo_t[oc*4+hh, src*128+p, c] =
    #   W_o[oc*1024+c, (src*4+hh)*128+p]
    woT = np.ascontiguousarray(W_o.T)  # [h', o]
    wo_t = np.empty((NOC * 4, 8 * P, OCC), dtype=bf)
    for oc in range(NOC):
        for hh in range(NH_LOC):
            for src in range(N_CORES):
                g = src * NH_LOC + hh
                wo_t[oc * 4 + hh, src * P:(src + 1) * P, :] = \
                    woT[g * P:(g + 1) * P, oc * OCC:(oc + 1) * OCC].astype(bf)

    in_maps = []
    for c in range(N_CORES):
        cols = []
        for hh in range(NH_LOC):
            q_rows = W_pack[c * VC + hh * P:c * VC + (hh + 1) * P]
            k_rows = W_pack[H + c * VC + hh * P:H + c * VC + (hh + 1) * P]
            cols.append(q_rows)
            cols.append(k_rows)
        wqk = np.concatenate(cols, axis=0)  # [1024, H] rows in pt order
        wqk_t = np.ascontiguousarray(wqk.T).astype(bf)  # [H, 1024]
        wtv_t = np.ascontiguousarray(
            W_pack[2 * H + c * VC:2 * H + (c + 1) * VC].T).astype(bf)
        in_maps.append({
            "hid_t": hid_t,
            "wqk_t": wqk_t,
            "wtv_t": wtv_t,
            "wo_t": wo_t,
            "cos_t": cos_t,
            "sin_t": sin_t,
        })
    return in_maps


_NC_CACHE = None


def get_nc():
    global _NC_CACHE
    if _NC_CACHE is None:
        _NC_CACHE = build_nc()
    return _NC_CACHE


def run(inputs, trace=False):
    """Run on hardware; returns (output [1,S,H] f32, BassKernelResults)."""
    in_maps = make_in_maps(
        inputs["hidden_states"], inputs["position_ids"],
        inputs["W_pack"], inputs["W_o"])
    nc = get_nc()
    res = run_bass_kernel_spmd(nc, in_maps, list(range(N_CORES)), trace=trace)
    parts = [np.asarray(res.results[c]["out_s"]) for c in range(N_CORES)]
    out = np.concatenate(parts, axis=0)[None]  # [1, S, H]
    return out.astype(np.float32), res


def kernel(**inputs):
    out, _ = run(inputs, trace=False)
    return out
